# revision 9
# baseline (speedup 1.0000x reference)
"""Trainium2 Bass kernel for a dense transformer block (attention + MLP).

Sharding: data-parallel over batch. 16 batch elements / 8 cores = 2 per core.
Each core runs the full block on its [2, 1024, 768] shard; no collectives.

v2 design (per core, T = 2048 tokens):
  - Activations feature-major ("L2": [feature partitions, token free]); all
    GEMMs are matmul(psum, lhsT=W[in,out], rhs=act[in,tok]) with fp32 PSUM.
  - QKV / proj / fc1 / fc2 run in fp8e4 (e4m3) with DoubleRow perf mode:
    contraction packed [128, 2, *]; weights pre-scaled x8 (x16 for fc2) on
    host to clear e4m3's subnormal floor, un-scaled in the PSUM evac.
    Attention (scores, AV) stays bf16.
  - LayerNorm affine folded into following weights on host. Stats via
    ones-vector matmuls on the PE; xhat / v-GEMM / proj / LN2 / MLP are
    pipelined per 512-token chunk so PE never waits on DVE-side LN work.
  - Softmax: scores built [key, query]; exp on ACT; denominator from a
    ones-column appended to V (row 64 of the ctx PSUM tile). Attention
    processes each 512-query chunk end-to-end: 4 score banks + 4 ctx
    accumulator banks fit the 8-bank PSUM exactly.
  - proj/fc1/fc2 weight DMAs issue before attention so they land during it.
"""

import numpy as np
import ml_dtypes

EMBED = 768
HIDDEN = 3072
HEADS = 12
HD = 64
EPS = 1e-6
B_FULL = 16
SEQ = 1024
NCORES = 8
BPC = B_FULL // NCORES          # batch elements per core
T = BPC * SEQ                   # tokens per core
KC = EMBED // 128               # 6 embed 128-chunks
KG = EMBED // 256               # 3 embed 256-groups (DoubleRow)
KT_H = HIDDEN // 128            # 24 hidden 128-chunks
KG_H = HIDDEN // 256            # 12 hidden 256-groups
MQK = 12                        # q+k output tiles (6 q, 6 k)
NT = T // 512                   # 4 token chunks of 512
TT = T // 128                   # 16 token tiles of 128
UT = SEQ // 128                 # 8 key tiles per batch

WS = 8.0                        # weight prescale (qkv/proj/fc1)
WS2 = 16.0                      # fc2 weight prescale

_CACHE = {}


def _build_nc(reps=1):
    import concourse.bass as bass
    import concourse.tile as tile
    from concourse import bacc, mybir
    from contextlib import ExitStack

    f32 = mybir.dt.float32
    bf16 = mybir.dt.bfloat16
    f8 = mybir.dt.float8e4
    AF = mybir.ActivationFunctionType
    OP = mybir.AluOpType
    DR = mybir.MatmulPerfMode.DoubleRow

    nc = bacc.Bacc()

    xT = nc.declare_dram_parameter("xT", [EMBED, T], f32, isOutput=False)
    wqkv = nc.declare_dram_parameter("wqkv", [KG * 128, 2 * 3 * EMBED], f8, isOutput=False)
    bqk = nc.declare_dram_parameter("bqk", [MQK, 128], f32, isOutput=False)
    bv = nc.declare_dram_parameter("bv", [EMBED], f32, isOutput=False)
    wproj = nc.declare_dram_parameter("wproj", [KG * 128, 2 * EMBED], f8, isOutput=False)
    bproj = nc.declare_dram_parameter("bproj", [KC, 128], f32, isOutput=False)
    w1d = nc.declare_dram_parameter("w1", [KG * 128, 2 * HIDDEN], f8, isOutput=False)
    b1d = nc.declare_dram_parameter("b1", [KT_H, 128], f32, isOutput=False)
    w2d = nc.declare_dram_parameter("w2", [KG_H * 128, 2 * EMBED], f8, isOutput=False)
    b2d = nc.declare_dram_parameter("b2", [KC, 128], f32, isOutput=False)
    outT = nc.declare_dram_parameter("outT", [EMBED, T], f32, isOutput=True)
    scratch = [nc.dram_tensor(f"scratch{i}", [EMBED, T], f32) for i in range(2)] if reps > 1 else []

    with tile.TileContext(nc) as tc, ExitStack() as es_glob:
        singles = es_glob.enter_context(tc.tile_pool(name="singles", bufs=1))

        ones_col = singles.tile([128, 1], bf16)
        nc.vector.memset(ones_col, 1.0)
        eps_sb = singles.tile([128, 1], f32)
        nc.vector.memset(eps_sb, EPS)

        bqk_sb = singles.tile([128, MQK], f32)
        nc.sync.dma_start(out=bqk_sb, in_=bqk.rearrange("t p -> p t"))
        bproj_sb = singles.tile([128, KC], f32)
        nc.sync.dma_start(out=bproj_sb, in_=bproj.rearrange("t p -> p t"))
        b1_sb = singles.tile([128, KT_H], f32)
        nc.sync.dma_start(out=b1_sb, in_=b1d.rearrange("t p -> p t"))
        b2_sb = singles.tile([128, KC], f32)
        nc.sync.dma_start(out=b2_sb, in_=b2d.rearrange("t p -> p t"))
        # v bias broadcast across all partitions (features live on free dim)
        bv_sb = singles.tile([128, EMBED], f32)
        bvap = bv[:]
        nc.sync.dma_start(
            out=bv_sb,
            in_=bass.AP(tensor=bvap.tensor, offset=bvap.offset,
                        ap=[[0, 128]] + list(bvap.ap)),
        )

        for _rep in range(reps):
            xT_in = xT if _rep == 0 else scratch[(_rep - 1) % 2]
            out_d = outT if _rep == reps - 1 else scratch[_rep % 2]

            # LEFT stack: ctx8 (lives A..D), then v/qk (live A..C), then the
            # per-phase pools nested inside. RIGHT stack: wqkv+xh (A..B),
            # then proj/fc1/fc2 weights + r1 + xh2 (B..end).
            es_rep = ExitStack()
            ctx_pool = es_rep.enter_context(tc.tile_pool(name="ctx", bufs=1))
            es_vqk = ExitStack()
            v_pool = es_vqk.enter_context(tc.tile_pool(name="v", bufs=1))
            qk_pool = es_vqk.enter_context(tc.tile_pool(name="qk", bufs=1))
            es_qkw = ExitStack()
            wq_pool = es_qkw.enter_context(tc.tile_pool(name="wqkv", bufs=1, side="right"))
            xh_pool = es_qkw.enter_context(tc.tile_pool(name="xh", bufs=1, side="right"))

            # ctx in fp8 DoubleRow layout for the proj GEMM; head h occupies
            # rows (h*64)%128 of group (h*64)//256, sub ((h*64)%256)//128.
            ctx8 = [ctx_pool.tile([128, 2, T], f8, tag=f"c{g}", name=f"ctx{g}")
                    for g in range(KG)]
            v_sb = [v_pool.tile([128, HEADS, HD + 1], bf16, tag=f"v{t}", name=f"v{t}")
                    for t in range(TT)]
            wq_sb = [wq_pool.tile([128, 2, 3 * EMBED], f8, tag=f"w{g}", name=f"wq{g}")
                     for g in range(KG)]
            xh8 = [[xh_pool.tile([128, 2, 512], f8, tag=f"xh{g}_{n}", name=f"xh{g}_{n}")
                    for n in range(NT)] for g in range(KG)]

            # ---------------- Phase A: LN1 + v GEMM, per-chunk pipelined ----
            es_a = ExitStack()
            xpool = es_a.enter_context(tc.tile_pool(name="x", bufs=8))
            xbpool = es_a.enter_context(tc.tile_pool(name="xb", bufs=1))
            sq_pool = es_a.enter_context(tc.tile_pool(name="sq1", bufs=3))
            st_pool = es_a.enter_context(tc.tile_pool(name="st1", bufs=2))
            bc_pool = es_a.enter_context(tc.tile_pool(name="bc1", bufs=1))
            tmp_pool = es_a.enter_context(tc.tile_pool(name="lntmp1", bufs=3))
            psln_pool = es_a.enter_context(tc.tile_pool(name="psln1", bufs=1, space="PSUM"))
            psv_pool = es_a.enter_context(tc.tile_pool(name="psv", bufs=2, space="PSUM"))
            psqk_pool = es_a.enter_context(tc.tile_pool(name="psqk", bufs=2, space="PSUM"))

            xb = [[xbpool.tile([128, 512], bf16, tag=f"xb{kc}_{n}", name=f"xb{kc}_{n}")
                   for n in range(NT)] for kc in range(KC)]

            def load_x(kc, n):
                xs = xpool.tile([128, 512], f32, tag="x", name=f"x{kc}_{n}")
                nc.sync.dma_start(
                    out=xs, in_=xT_in[kc * 128:(kc + 1) * 128, n * 512:(n + 1) * 512])
                nc.vector.tensor_copy(xb[kc][n], xs)

            # x chunk 0 first, then qkv weights, then the rest: the first
            # stats matmuls only need chunk 0; wqkv is needed by the v GEMM
            # that follows xhat(0).
            for kc in range(KC):
                load_x(kc, 0)
            for g in range(KG):
                nc.sync.dma_start(out=wq_sb[g], in_=wqkv[g * 128:(g + 1) * 128, :])
            for n in range(1, NT):
                for kc in range(KC):
                    load_x(kc, n)

            rb = [bc_pool.tile([128, 512], bf16, tag=f"rb{n}", name=f"rb{n}")
                  for n in range(NT)]
            nmrb = [bc_pool.tile([128, 512], bf16, tag=f"nmrb{n}", name=f"nmrb{n}")
                    for n in range(NT)]

            def ln_stats(n, src, rb_n, nmrb_n, ps_pool, stp, sqp, sfx):
                """src[kc]: [128,512] bf16 tiles. Emits stats for token chunk
                n -> partition-broadcast tiles rb_n (rstd), nmrb_n (-mu*rstd).
                Cross-partition sums via ones matmuls on the PE."""
                ps_sum = ps_pool.tile([1, 512], f32, tag="psum", name=f"psum{sfx}{n}")
                ps_sq = ps_pool.tile([1, 512], f32, tag="psq", name=f"psq{sfx}{n}")
                for kc in range(KC):
                    sqt = sqp.tile([128, 512], bf16, tag="sq",
                                   name=f"sq{sfx}_{n}_{kc}", bufs=3)
                    nc.scalar.activation(out=sqt, in_=src[kc], func=AF.Square)
                    nc.tensor.matmul(ps_sum, ones_col, src[kc],
                                     start=(kc == 0), stop=(kc == KC - 1))
                    nc.tensor.matmul(ps_sq, ones_col, sqt,
                                     start=(kc == 0), stop=(kc == KC - 1))
                mean = stp.tile([1, 512], f32, tag="mean", name=f"mean{sfx}{n}", bufs=2)
                var = stp.tile([1, 512], f32, tag="var", name=f"var{sfx}{n}", bufs=2)
                rstd = stp.tile([1, 512], f32, tag="rstd", name=f"rstd{sfx}{n}", bufs=2)
                nmr = stp.tile([1, 512], f32, tag="nmr", name=f"nmr{sfx}{n}", bufs=2)
                stb0 = stp.tile([1, 512], bf16, tag="stb0", name=f"stb0{sfx}{n}", bufs=2)
                stb1 = stp.tile([1, 512], bf16, tag="stb1", name=f"stb1{sfx}{n}", bufs=2)
                nc.vector.tensor_scalar_mul(mean, ps_sum, 1.0 / EMBED)
                # nmr temporarily holds -mean^2
                nc.vector.scalar_tensor_tensor(
                    out=nmr, in0=mean, scalar=-1.0, in1=mean,
                    op0=OP.mult, op1=OP.mult)
                nc.vector.scalar_tensor_tensor(
                    out=var, in0=ps_sq, scalar=1.0 / EMBED, in1=nmr,
                    op0=OP.mult, op1=OP.add)
                nc.scalar.activation(out=var, in_=var, func=AF.Sqrt,
                                     bias=eps_sb[0:1, :])
                nc.vector.reciprocal(rstd, var)
                nc.vector.scalar_tensor_tensor(
                    out=nmr, in0=mean, scalar=-1.0, in1=rstd,
                    op0=OP.mult, op1=OP.mult)
                nc.vector.tensor_copy(stb0, rstd)
                nc.vector.tensor_copy(stb1, nmr)
                nc.gpsimd.partition_broadcast(rb_n, stb0)
                nc.gpsimd.partition_broadcast(nmrb_n, stb1)

            for n in range(NT):
                ln_stats(n, [xb[kc][n] for kc in range(KC)],
                         rb[n], nmrb[n], psln_pool, st_pool, sq_pool, "1")

            q_sb, k_sb = {}, {}
            for i in range(KC):
                q_sb[i] = qk_pool.tile([128, T], bf16, tag=f"qk{i}", name=f"qk{i}")
                k_sb[i] = qk_pool.tile([128, T], bf16, tag=f"qk{KC + i}", name=f"qk{KC + i}")

            # xhat (fp8, DoubleRow layout) + v GEMM + q/k GEMM per chunk
            for n in range(NT):
                for kc in range(KC):
                    tmp = tmp_pool.tile([128, 512], bf16, tag="t", name=f"lntmp1_{n}_{kc}")
                    nc.vector.tensor_tensor(tmp, xb[kc][n], rb[n], OP.mult)
                    nc.vector.tensor_tensor(
                        xh8[kc // 2][n][:, kc % 2, :], tmp, nmrb[n], OP.add)
                for t in range(4 * n, 4 * n + 4):
                    vt = v_sb[t]
                    nc.vector.memset(vt[:, :, HD:HD + 1], 1.0)
                    psv = psv_pool.tile([128, EMBED], f32, tag="psv", name=f"psv{t}")
                    tsl = slice((t % 4) * 128, (t % 4) * 128 + 128)
                    for g in range(KG):
                        for lo, hi in ((0, 512), (512, EMBED)):
                            nc.tensor.matmul(
                                psv[:, lo:hi],
                                xh8[g][n][:, :, tsl],
                                wq_sb[g][:, :, 2 * EMBED + lo:2 * EMBED + hi],
                                start=(g == 0),
                                stop=(g == KG - 1),
                                perf_mode=DR,
                            )
                    nc.vector.scalar_tensor_tensor(
                        out=vt[:, :, 0:HD],
                        in0=psv.rearrange("p (h d) -> p h d", h=HEADS),
                        scalar=1.0 / WS,
                        in1=bv_sb.rearrange("p (h d) -> p h d", h=HEADS),
                        op0=OP.mult, op1=OP.add,
                    )
                # q/k for this token chunk (evac on ACT: DVE owns xhat here)
                sl = slice(n * 512, (n + 1) * 512)
                for m in range(MQK):
                    qkt = q_sb[m] if m < KC else k_sb[m - KC]
                    escale = (1.0 / (WS * 8.0)) if m < KC else (1.0 / WS)
                    ps = psqk_pool.tile([128, 512], f32, tag="ps",
                                        name=f"psqk{m}_{n}")
                    for g in range(KG):
                        nc.tensor.matmul(
                            ps,
                            wq_sb[g][:, :, m * 128:(m + 1) * 128],
                            xh8[g][n],
                            start=(g == 0),
                            stop=(g == KG - 1),
                            perf_mode=DR,
                        )
                    nc.scalar.activation(
                        out=qkt[:, sl], in_=ps, func=AF.Identity,
                        bias=bqk_sb[:, m:m + 1], scale=escale)
            es_a.close()  # frees x ring, xb, LN1 stats, broadcasts
            es_qkw.close()  # frees wqkv + xh8 (right stack empty)

            # prefetch proj/fc1/fc2 weights; they land during attention
            es_wR = ExitStack()
            wp_pool = es_wR.enter_context(tc.tile_pool(name="wp", bufs=1, side="right"))
            w1_pool = es_wR.enter_context(tc.tile_pool(name="w1", bufs=1, side="right"))
            w2_pool = es_wR.enter_context(tc.tile_pool(name="w2", bufs=1, side="right"))
            r1_pool = es_wR.enter_context(tc.tile_pool(name="r1", bufs=1, side="right"))
            xh2_pool = es_wR.enter_context(tc.tile_pool(name="xh2", bufs=1, side="right"))
            wp_sb = [wp_pool.tile([128, 2, EMBED], f8, tag=f"wp{g}", name=f"wp{g}")
                     for g in range(KG)]
            for g in range(KG):
                nc.sync.dma_start(out=wp_sb[g], in_=wproj[g * 128:(g + 1) * 128, :])
            w1_sb = [w1_pool.tile([128, 2, HIDDEN], f8, tag=f"a{g}", name=f"w1_{g}")
                     for g in range(KG)]
            for g in range(KG):
                nc.sync.dma_start(out=w1_sb[g], in_=w1d[g * 128:(g + 1) * 128, :])
            w2_sb = [w2_pool.tile([128, 2, EMBED], f8, tag=f"b{g}", name=f"w2_{g}")
                     for g in range(KG_H)]
            for g in range(KG_H):
                nc.sync.dma_start(out=w2_sb[g], in_=w2d[g * 128:(g + 1) * 128, :])

            # ---------------- Phase C: attention ----------------
            es_att = ExitStack()
            exp_pool = es_att.enter_context(tc.tile_pool(name="exp", bufs=4))
            rc_pool = es_att.enter_context(tc.tile_pool(name="rc", bufs=2))
            pssc_pool = es_att.enter_context(tc.tile_pool(name="pssc", bufs=2, space="PSUM"))
            psctx_pool = es_att.enter_context(tc.tile_pool(name="psctx", bufs=4, space="PSUM"))

            def attention_qc(i, b, qc):
                """Heads (2i, 2i+1), batch b, query chunk qc (512 queries).
                Even head on array rows 0-63, odd on 64-127: score matmuls of
                the two heads hit disjoint row groups. Both heads' AV
                accumulate inside the ut loop; tails (recip/broadcast/evac)
                overlap the next chunk's scores."""
                h0, h1 = 2 * i, 2 * i + 1
                qt, kt = q_sb[i], k_sb[i]
                tok = slice(b * SEQ, (b + 1) * SEQ)
                qsl = slice(b * SEQ + qc * 512, b * SEQ + (qc + 1) * 512)
                cacc = {}
                for h in (h0, h1):
                    cacc[h] = psctx_pool.tile(
                        [HD + 1, 512], f32, tag="ctx", name=f"psc{h}_{b}_{qc}")

                def scores(ut):
                    # both heads' logits in one 2-bank tile: h0 -> cols
                    # 0:512 (bank A), h1 -> cols 512:1024 (bank B), so a
                    # single exp covers the pair.
                    ps = pssc_pool.tile([128, 1024], f32, tag="sc",
                                        name=f"sc{i}_{b}_{qc}_{ut}")
                    for h in (h0, h1):
                        r0 = (h % 2) * HD
                        nc.tensor.matmul(
                            ps[:, r0 * 8:r0 * 8 + 512],
                            kt[r0:r0 + HD, tok][:, ut * 128:(ut + 1) * 128],
                            qt[r0:r0 + HD, qsl],
                        )
                    return ps

                pss = scores(0)
                for ut in range(UT):
                    nxt = scores(ut + 1) if ut < UT - 1 else None
                    e = exp_pool.tile([128, 1024], bf16, tag="e",
                                      name=f"e{i}_{b}_{qc}_{ut}")
                    nc.scalar.activation(out=e, in_=pss, func=AF.Exp)
                    for h in (h0, h1):
                        r0 = (h % 2) * HD
                        nc.tensor.matmul(
                            cacc[h],
                            v_sb[b * UT + ut][:, h, :],
                            e[:, r0 * 8:r0 * 8 + 512],
                            start=(ut == 0),
                            stop=(ut == UT - 1),
                        )
                    pss = nxt

                # tails: denominators + evac into fp8 ctx
                for h in (h0, h1):
                    rc = rc_pool.tile([1, 512], f32, tag="rc", name=f"rc{h}_{b}_{qc}")
                    nc.vector.reciprocal(rc, cacc[h][HD:HD + 1, :])
                    rbh = rc_pool.tile([HD, 512], f32, tag="rb", name=f"rbh{h}_{b}_{qc}")
                    nc.gpsimd.partition_broadcast(rbh, rc)
                    g, sub = (h * HD) // 256, ((h * HD) % 256) // 128
                    r0 = (h * HD) % 128
                    nc.vector.tensor_tensor(
                        ctx8[g][r0:r0 + HD, sub, qsl],
                        cacc[h][0:HD, :], rbh, OP.mult)

            for i in range(KC):
                for b in range(BPC):
                    for qc in range(2):
                        attention_qc(i, b, qc)

            es_att.close()
            es_vqk.close()  # frees v + qk

            # -------- Phase D/E/F: proj + LN2 + MLP, per-chunk pipelined ----
            es_f = ExitStack()
            xr_pool = es_f.enter_context(tc.tile_pool(name="xr", bufs=1))
            pt_pool = es_f.enter_context(tc.tile_pool(name="pt", bufs=4))
            sq2_pool = es_f.enter_context(tc.tile_pool(name="sq2", bufs=3))
            st2_pool = es_f.enter_context(tc.tile_pool(name="st2", bufs=2))
            bc2_pool = es_f.enter_context(tc.tile_pool(name="bc2", bufs=1))
            tmp2_pool = es_f.enter_context(tc.tile_pool(name="lntmp2", bufs=3))
            h1_pool = es_f.enter_context(tc.tile_pool(name="h1", bufs=1))
            o_pool = es_f.enter_context(tc.tile_pool(name="o", bufs=3))
            psd_pool = es_f.enter_context(tc.tile_pool(name="psd", bufs=2, space="PSUM"))
            psln2_pool = es_f.enter_context(tc.tile_pool(name="psln2", bufs=1, space="PSUM"))
            ps1_pool = es_f.enter_context(tc.tile_pool(name="ps1", bufs=2, space="PSUM"))
            ps2_pool = es_f.enter_context(tc.tile_pool(name="ps2", bufs=2, space="PSUM"))

            r1 = [[r1_pool.tile([128, 512], bf16, tag=f"r{m}_{n}", name=f"r1_{m}_{n}")
                   for n in range(NT)] for m in range(KC)]
            rb2 = [bc2_pool.tile([128, 512], bf16, tag=f"rb2{n}", name=f"rb2{n}")
                   for n in range(NT)]
            nmrb2 = [bc2_pool.tile([128, 512], bf16, tag=f"nmrb2{n}", name=f"nmrb2{n}")
                     for n in range(NT)]
            xh28 = [[xh2_pool.tile([128, 2, 512], f8, tag=f"h{g}_{n}", name=f"xh2_{g}_{n}")
                     for n in range(NT)] for g in range(KG)]

            def proj_chunk(n):
                sl = slice(n * 512, (n + 1) * 512)
                for m in range(KC):
                    xr = xr_pool.tile([128, 512], f32, tag="xr", bufs=8,
                                      name=f"xr{m}_{n}")
                    nc.sync.dma_start(out=xr, in_=xT_in[m * 128:(m + 1) * 128, sl])
                    ps = psd_pool.tile([128, 512], f32, tag="ps", name=f"psd{m}_{n}")
                    for g in range(KG):
                        nc.tensor.matmul(
                            ps,
                            wp_sb[g][:, :, m * 128:(m + 1) * 128],
                            ctx8[g][:, :, sl],
                            start=(g == 0),
                            stop=(g == KG - 1),
                            perf_mode=DR,
                        )
                    pt = pt_pool.tile([128, 512], bf16, tag="pt", name=f"pt{m}_{n}")
                    nc.scalar.activation(
                        out=pt, in_=ps, func=AF.Identity,
                        bias=bproj_sb[:, m:m + 1], scale=1.0 / WS)
                    nc.vector.tensor_tensor(r1[m][n], pt, xr, OP.add)

            def ln2_chunk(n):
                ln_stats(n, [r1[kc][n] for kc in range(KC)],
                         rb2[n], nmrb2[n], psln2_pool, st2_pool, sq2_pool, "2")

            def xhat2_chunk(n):
                for kc in range(KC):
                    tmp = tmp2_pool.tile([128, 512], bf16, tag="t",
                                         name=f"lntmp2_{n}_{kc}")
                    nc.vector.tensor_tensor(tmp, r1[kc][n], rb2[n], OP.mult)
                    nc.vector.tensor_tensor(
                        xh28[kc // 2][n][:, kc % 2, :], tmp, nmrb2[n], OP.add)

            h18 = [h1_pool.tile([128, 2, 512], f8, tag=f"h1_{g}", name=f"h1g{g}")
                   for g in range(KG_H)]

            def fc1_chunk(n):
                xhat2_chunk(n)
                for kt in range(KT_H):
                    ps1 = ps1_pool.tile([128, 512], f32, tag="p1", name=f"ps1_{n}_{kt}")
                    for g in range(KG):
                        nc.tensor.matmul(
                            ps1,
                            w1_sb[g][:, :, kt * 128:(kt + 1) * 128],
                            xh28[g][n],
                            start=(g == 0),
                            stop=(g == KG - 1),
                            perf_mode=DR,
                        )
                    nc.scalar.activation(
                        out=h18[kt // 2][:, kt % 2, :], in_=ps1, func=AF.Gelu,
                        bias=b1_sb[:, kt:kt + 1], scale=1.0 / WS)

            def fc2_chunk(n):
                sl = slice(n * 512, (n + 1) * 512)
                for m in range(KC):
                    ps2 = ps2_pool.tile([128, 512], f32, tag="f", name=f"ps2_{n}_{m}")
                    for g in range(KG_H):
                        nc.tensor.matmul(
                            ps2,
                            w2_sb[g][:, :, m * 128:(m + 1) * 128],
                            h18[g],
                            start=(g == 0),
                            stop=(g == KG_H - 1),
                            perf_mode=DR,
                        )
                    ft = pt_pool.tile([128, 512], bf16, tag="ft", name=f"ft{m}_{n}")
                    nc.scalar.activation(
                        out=ft, in_=ps2, func=AF.Identity,
                        bias=b2_sb[:, m:m + 1], scale=1.0 / WS2)
                    ot = o_pool.tile([128, 512], f32, tag="o", name=f"ot{n}_{m}")
                    nc.vector.tensor_tensor(ot, ft, r1[m][n], OP.add)
                    nc.sync.dma_start(out=out_d[m * 128:(m + 1) * 128, sl], in_=ot)

            # pipelined emission: PE stream stays dense while LN2 stats/xhat
            # for chunk n+1 run on ACT/DVE/Pool under the MLP of chunk n.
            proj_chunk(0)
            proj_chunk(1)
            ln2_chunk(0)
            proj_chunk(2)
            ln2_chunk(1)
            fc1_chunk(0)
            fc2_chunk(0)
            ln2_chunk(2)
            proj_chunk(3)
            fc1_chunk(1)
            ln2_chunk(3)
            fc2_chunk(1)
            fc1_chunk(2)
            fc2_chunk(2)
            fc1_chunk(3)
            fc2_chunk(3)
            es_f.close()
            es_wR.close()
            es_rep.close()

    nc.compile()
    return nc


def _prep_host(inputs):
    """Fold LN affine into weights; quantize weights to prescaled fp8 in the
    DoubleRow [K/256, 128, 2, M] layout; transpose/cast for the kernel."""
    f32 = np.float32
    f8 = ml_dtypes.float8_e4m3

    def dr_pack(wT, scale):
        # wT: [K, M] fp32 (already transposed, contraction-major).
        K, M = wT.shape
        w = (wT * scale).reshape(K // 256, 2, 128, M).transpose(0, 2, 1, 3)
        return np.ascontiguousarray(w.reshape(K // 256 * 128, 2 * M)).astype(f8)

    g1 = inputs["ln1_g"].astype(f32)
    b1n = inputs["ln1_b"].astype(f32)
    g2 = inputs["ln2_g"].astype(f32)
    b2n = inputs["ln2_b"].astype(f32)

    wqkv = np.asarray(inputs["qkv_w"], dtype=f32)          # [2304, 768]
    bqkv = np.asarray(inputs["qkv_b"], dtype=f32)
    w_eff = wqkv * g1[None, :]
    b_eff = bqkv + wqkv @ b1n
    s = 1.0 / np.sqrt(HD)
    b_eff[:EMBED] *= s  # q bias gets the attention scale (weights don't;
    # the 1/8 rides in the q evac scale instead)

    w1 = np.asarray(inputs["fc1_w"], dtype=f32)            # [3072, 768]
    b1 = np.asarray(inputs["fc1_b"], dtype=f32)
    w1_eff = w1 * g2[None, :]
    b1_eff = b1 + w1 @ b2n

    return {
        "wqkv": dr_pack(np.ascontiguousarray(w_eff.T), WS),
        "bqk": np.ascontiguousarray(b_eff[:2 * EMBED].reshape(MQK, 128)),
        "bv": np.ascontiguousarray(b_eff[2 * EMBED:]),
        "wproj": dr_pack(np.ascontiguousarray(np.asarray(inputs["proj_w"], dtype=f32).T), WS),
        "bproj": np.ascontiguousarray(np.asarray(inputs["proj_b"], dtype=f32).reshape(KC, 128)),
        "w1": dr_pack(np.ascontiguousarray(w1_eff.T), WS),
        "b1": np.ascontiguousarray(b1_eff.reshape(KT_H, 128)),
        "w2": dr_pack(np.ascontiguousarray(np.asarray(inputs["fc2_w"], dtype=f32).T), WS2),
        "b2": np.ascontiguousarray(np.asarray(inputs["fc2_b"], dtype=f32).reshape(KC, 128)),
    }


def kernel(**inputs) -> np.ndarray:
    from concourse import bass_utils

    if "nc" not in _CACHE:
        _CACHE["nc"] = _build_nc()
    nc = _CACHE["nc"]

    shared = _prep_host(inputs)
    x = np.asarray(inputs["x"], dtype=np.float32)  # [16, 1024, 768]
    in_maps = []
    for c in range(NCORES):
        xc = x[c * BPC:(c + 1) * BPC].reshape(T, EMBED)
        in_maps.append({"xT": np.ascontiguousarray(xc.T), **shared})

    res = bass_utils.run_bass_kernel_spmd(nc, in_maps, list(range(NCORES)))
    outs = []
    for c in range(NCORES):
        oT = res.results[c]["outT"]  # [768, T]
        outs.append(oT.T.reshape(BPC, SEQ, EMBED))
    return np.concatenate(outs, axis=0).astype(np.float32)
